# revision 18
# baseline (speedup 1.0000x reference)
"""BiaffineAttn Trainium2 kernel.

Math (per batch b):
    t    = x2 @ U + bias[None, :]      [S, D]   (bias folded: x2 U x1^T + 1 (x1 bias)^T
                                                 == [x2|1] [[U];[bias^T]] x1^T)
    attn = t @ x1^T
    p    = softmax(attn, axis=-1)
    out  = relu((p @ x1) @ fc_w^T + fc_b)       [S, F]

Sharding: data-parallel over batch B=8, one batch per NeuronCore.

Per-core pipeline, transposed orientation (softmax key dim t' on partitions),
software-pipelined across S-superblocks so the PE runs pure GEMM streams only:
  PE:     MM1 / MM2 / MM4 / MM5 — 352 x 512-col fp16 streams per superblock,
          no transposes, no rank-1 sums, no broadcast matmuls
  Scalar: exp(sb, tile) + the MM2 psum->SBUF score copy (+ MM1 tt copies)
  DVE:    score running-max + maxb subtract, reciprocal, psum->SBUF copies
  Pool:   softmax partition stats — per-s max via partition_all_reduce(max),
          denominator via in-place fp32 add chains over the 16 pb tiles +
          partition_all_reduce(add); both off the PE critical path

All 16-bit matmuls run in fp16 (TRN2 forbids mixing 16/32-bit matmul inputs;
fp16 keeps score noise ~2^-11-scale and LDWEIGHTS at 97ns so the stationary
load hides under the 213ns stream). reciprocal_approx_fast for the denom;
relu+bias via Scalar activation; [F,SB] stores per superblock.

All tensors are host-relaid so every resident loads in ONE big DMA (32KB
rows); x1t t'-groups go on the Pool SWDGE queue (idle during the prologue)
so the scalar HWDGE queue carries only the x2t stream.

Host side: builds the relaid views and transposes the [F,S] per-core output
back to [S,F] when gathering (fp32 DMA transpose does not exist on TRN2).
"""

import os
import sys
from contextlib import ExitStack

import numpy as np

for _p in ("/opt/trn_rl_repo", os.path.expanduser("~/.axon_site/_ro/trn_rl_repo")):
    if os.path.isdir(_p) and _p not in sys.path:
        sys.path.insert(0, _p)

import concourse.bass as bass
import concourse.mybir as mybir
import concourse.tile as tile
from concourse import bacc, bass_isa

B = 8
S = 2048          # sequence length (both s and t')
D = 1024          # d_model
F = 512           # fc output dim
P = 128
SB = 512          # s superblock (moving free dim of every matmul)
NSB = S // SB     # 4
DC = D // P       # 8 contraction chunks of d / e
TC = S // P       # 16 t' tiles
NTG = NSB         # 4 t' groups of 4 tiles
FT = F // P       # 4
FP32 = mybir.dt.float32
BF16 = mybir.dt.bfloat16
FP16 = mybir.dt.float16
AF = mybir.ActivationFunctionType
ALU = mybir.AluOpType
AX = mybir.AxisListType

OT_DT = FP16      # dtype of MM4 output tiles (MM5 moving operand)


def build_nc():
    nc = bacc.Bacc(
        "TRN2",
        target_bir_lowering=False,
        debug=False,
        enable_asserts=False,
    )

    # host-relaid tensors: row p holds the p-th partition's data for every tile
    x1_d = nc.dram_tensor("x1g", [P, TC * D], FP16, kind="ExternalInput")
    x1t_d = nc.dram_tensor("x1tg", [P, NTG * DC * SB], FP16, kind="ExternalInput")
    x2t_d = nc.dram_tensor("x2t", [D, S], FP16, kind="ExternalInput")
    u_d = nc.dram_tensor("ug", [P, DC * D], FP16, kind="ExternalInput")
    fcwt_d = nc.dram_tensor("fcwg", [P, DC * F], FP16, kind="ExternalInput")
    bias_d = nc.dram_tensor("biasg", [P, DC], FP32, kind="ExternalInput")
    fcb_d = nc.dram_tensor("fcbg", [P, FT], FP32, kind="ExternalInput")
    outt_d = nc.dram_tensor("outt", [F, S], FP32, kind="ExternalOutput")

    with tile.TileContext(nc) as tc, ExitStack() as ctx:
        # ---------- pools ----------
        p_u = ctx.enter_context(tc.tile_pool(name="ures", bufs=1))
        p_x1 = ctx.enter_context(tc.tile_pool(name="x1res", bufs=1))
        p_bc = ctx.enter_context(tc.tile_pool(name="biascols", bufs=1))
        p_fcb = ctx.enter_context(tc.tile_pool(name="fcbcols", bufs=1))
        p_fcw = ctx.enter_context(tc.tile_pool(name="fcwres", bufs=1))
        p_psum = ctx.enter_context(tc.tile_pool(name="psum", bufs=4, space="PSUM"))
        p_psum1 = ctx.enter_context(tc.tile_pool(name="psum1", bufs=4, space="PSUM"))
        p_x2t = ctx.enter_context(tc.tile_pool(name="x2ts", bufs=12))
        p_x1tc = ctx.enter_context(tc.tile_pool(name="x1tgs", bufs=4))
        p_tt = ctx.enter_context(tc.tile_pool(name="tts", bufs=9))
        p_sc = ctx.enter_context(tc.tile_pool(name="scores", bufs=TC))
        p_pb = ctx.enter_context(tc.tile_pool(name="pbf", bufs=TC))
        p_ot = ctx.enter_context(tc.tile_pool(name="ots", bufs=12))
        p_aux = ctx.enter_context(tc.tile_pool(name="aux", bufs=1))
        p_den = ctx.enter_context(tc.tile_pool(name="dens", bufs=1))
        p_tmp = ctx.enter_context(tc.tile_pool(name="tmps", bufs=2))
        p_oo = ctx.enter_context(tc.tile_pool(name="oos", bufs=4))

        # ---------- prologue DMAs: MM1(0) critical path first ----------
        # x1t resident (4MB fp16 = the whole tensor): one SWDGE DMA per
        # t'-group on the otherwise-idle Pool queue, so the scalar HWDGE
        # queue carries only the x2t stream and tg0 lands before MM2(0) ti0
        x1tg = {}

        def issue_x1tg(key, tg, eng=None):
            eng = eng or nc.gpsimd
            t = p_x1tc.tile([P, DC * SB], FP16, name=f"x1tg{key}", tag="x1tg")
            eng.dma_start(t[:], x1t_d[:, tg * DC * SB : (tg + 1) * DC * SB])
            x1tg[key] = t

        # per-dc pieces: each lands just ahead of MM1(0)'s dc step;
        # tg2/tg3 queue on sync behind u (deadlines MM2(0) ti8/ti12)
        u_big = p_u.tile([P, DC * D], FP16, name="ug", tag="ur")
        for q in range(4):
            nc.sync.dma_start(
                u_big[:, q * (D // 4) : (q + 1) * (D // 4)],
                u_d[:, q * (D // 4) : (q + 1) * (D // 4)],
            )
        for dc in range(1, DC):
            nc.sync.dma_start(
                u_big[:, dc * D : (dc + 1) * D], u_d[:, dc * D : (dc + 1) * D]
            )
        issue_x1tg(0, 0, nc.sync)
        issue_x1tg(1, 1, nc.sync)
        issue_x1tg(2, 2, nc.sync)
        issue_x1tg(3, 3, nc.sync)

        def u_sl(dc, et):
            return u_big[:, dc * D + et * P : dc * D + (et + 1) * P]

        # scalar HWDGE queue: x2t(0) then bias then x2t(1)
        bias_cols = p_bc.tile([P, DC], FP32, name="bc", tag="bc")
        nc.scalar.dma_start(bias_cols[:], bias_d[:, :])
        x2t_tiles = {0: []}
        for dc in range(DC):
            x2_t = p_x2t.tile([P, SB], FP16, name=f"x2t0_{dc}", tag="x2t")
            nc.scalar.dma_start(x2_t[:], x2t_d[dc * P : (dc + 1) * P, 0:SB])
            x2t_tiles[0].append(x2_t)

        def issue_x2t(sb, eng=None):
            eng = eng or nc.sync
            tiles = []
            for dc in range(DC):
                x2_t = p_x2t.tile([P, SB], FP16, name=f"x2t{sb}_{dc}", tag="x2t")
                eng.dma_start(
                    x2_t[:], x2t_d[dc * P : (dc + 1) * P, sb * SB : (sb + 1) * SB]
                )
                tiles.append(x2_t)
            return tiles

        x2t_tiles[1] = issue_x2t(1, nc.scalar)

        # residents declared here; DMAs issued after prologue compute emission
        # (needed only from MM4(0)/MM5(0) on — keep them off the critical
        # 20-40us DMA window)
        x1_big = p_x1.tile([P, TC * D], FP16, name="x1g", tag="x1r")

        def x1_sl(ti, et):
            return x1_big[:, ti * D + et * P : ti * D + (et + 1) * P]

        fcw_big = p_fcw.tile([P, DC * F], FP16, name="fcwg", tag="fcw")

        def fcw_sl(ec, ft):
            return fcw_big[:, ec * F + ft * P : ec * F + (ft + 1) * P]

        fcb_cols = p_fcb.tile([P, FT], FP32, name="fcb", tag="fcb")

        ones_colb = p_bc.tile([P, 1], FP16, name="ones_colb", tag="ones_c")
        nc.gpsimd.memset(ones_colb[:], 1.0)
        ones_row32 = p_bc.tile([1, P], FP32, name="ones_row32", tag="ones_r")
        nc.gpsimd.memset(ones_row32[:], 1.0)

        # ---------- MM1: ttT = (x2 @ U)^T + bias, 8 held banks (both pools) ----------
        tt = {}

        def emit_mm1(sb, streaming=False, dve_copies=False):
            tt[sb] = []
            if streaming:
                # dc-outer, 8 held banks: consumes each u/x2t piece once as it
                # lands (prologue is DMA-paced)
                ps = [
                    p_psum1.tile([P, SB], FP32, name=f"ps1_{sb}_{et}", tag="ps1")
                    for et in range(4)
                ] + [
                    p_psum.tile([P, SB], FP32, name=f"ps1b_{sb}_{et}", tag="ps")
                    for et in range(4, DC)
                ]
                for dc in range(DC):
                    for et in range(DC):
                        nc.tensor.matmul(
                            ps[et][:], u_sl(dc, et), x2t_tiles[sb][dc][:],
                            start=(dc == 0), stop=(dc == DC - 1),
                        )
                for et in range(DC):
                    t_t = p_tt.tile([P, SB], FP16, name=f"tt{sb}_{et}", tag="tt")
                    nc.scalar.activation(
                        t_t[:], ps[et][:], AF.Identity,
                        bias=bias_cols[:, et : et + 1], scale=1.0,
                    )
                    tt[sb].append(t_t)
                return
            # et-outer, one rotating bank: copies spread across MM1 so the
            # scalar queue never piles up at the block seam
            for et in range(DC):
                ps = p_psum1.tile([P, SB], FP32, name=f"ps1_{sb}_{et}", tag="ps1")
                for dc in range(DC):
                    nc.tensor.matmul(
                        ps[:], u_sl(dc, et), x2t_tiles[sb][dc][:],
                        start=(dc == 0), stop=(dc == DC - 1),
                    )
                t_t = p_tt.tile([P, SB], FP16, name=f"tt{sb}_{et}", tag="tt")
                if dve_copies:
                    # DVE bias-fused copy: keeps the scalar queue free for the
                    # surrounding MM2 sc-copies (prologue-only, avoids deadlock)
                    nc.vector.tensor_scalar_add(
                        t_t[:], ps[:], bias_cols[:, et : et + 1]
                    )
                else:
                    nc.scalar.activation(
                        t_t[:], ps[:], AF.Identity,
                        bias=bias_cols[:, et : et + 1], scale=1.0,
                    )
                tt[sb].append(t_t)

        emit_mm1(0, streaming=True)

        # ---------- MM2 per-tile emitter: scoresT tile + running max ----------
        sc_tiles = {}
        maxacc = {}
        pb_tiles = {}

        def emit_mm2_tile(sb, ti):
            tg, sub = divmod(ti, SB // P)
            if ti == 0:
                sc_tiles[sb] = []
                maxacc[sb] = p_aux.tile(
                    [P, SB], FP32, name=f"maxacc{sb}", tag=f"maxacc{sb % 2}"
                )
            ps_s = p_psum.tile([P, SB], FP32, name=f"pss{sb}_{ti}", tag="ps")
            grp = x1tg[tg]
            for ec in range(DC):
                nc.tensor.matmul(
                    ps_s[:],
                    grp[:, ec * SB + sub * P : ec * SB + (sub + 1) * P],
                    tt[sb][ec][:],
                    start=(ec == 0), stop=(ec == DC - 1),
                )
            s_t = p_sc.tile([P, SB], FP32, name=f"sc{sb}_{ti}", tag="sc")
            nc.scalar.copy(s_t[:], ps_s[:])
            if ti == 0:
                nc.vector.tensor_copy(maxacc[sb][:], s_t[:])
            else:
                nc.vector.tensor_max(maxacc[sb][:], maxacc[sb][:], s_t[:])
            sc_tiles[sb].append(s_t)

        # prologue MM2(0) tg0, then MM1(1) (tt copies on DVE so the scalar
        # queue keeps draining MM2 sc-copies), then MM2(0) tg1-3: pushes the
        # tg1-3 DMA deadlines out by MM1(1)'s 14us
        for ti in range(4):
            emit_mm2_tile(0, ti)
        emit_mm1(1, dve_copies=True)
        for ti in range(4, TC):
            emit_mm2_tile(0, ti)
        nc.sync.dma_start(x1_big[:, : TC * D // 2], x1_d[:, : TC * D // 2])
        nc.sync.dma_start(x1_big[:, TC * D // 2 :], x1_d[:, TC * D // 2 :])
        nc.sync.dma_start(fcw_big[:], fcwt_d[:, :])
        nc.sync.dma_start(fcb_cols[:], fcb_d[:, :])
        maxbs = {}

        # ---------- softmax helpers (all partition stats on Pool) ----------
        def emit_maxb(sb):
            """per-s max over partitions, broadcast back: one Pool op."""
            maxb = p_aux.tile([P, SB], FP32, name=f"maxb{sb}", tag=f"maxb{sb % 2}")
            nc.gpsimd.partition_all_reduce(
                maxb[:], maxacc[sb][:], channels=P,
                reduce_op=bass_isa.ReduceOp.max,
            )
            return maxb

        def emit_exp_tile(sb, ti, maxb):
            if ti == 0:
                pb_tiles[sb] = []
            s_t = sc_tiles[sb][ti]
            # scheduling hint only (no HW wait): keep window sb's subs behind
            # window sb-1's ot-CASTs in the DVE queue — the list scheduler's
            # Pool cost model is optimistic and would otherwise hoist them,
            # stalling MM4's PSUM-bank recycling on real hardware
            with tc.tile_wait_until(0.060 + 0.080 * sb):
                nc.vector.tensor_sub(s_t[:], s_t[:], maxb[:])
            p_t = p_pb.tile([P, SB], FP16, name=f"pb{sb}_{ti}", tag="pb")
            nc.scalar.activation(p_t[:], s_t[:], AF.Exp, bias=0.0, scale=1.0)
            pb_tiles[sb].append(p_t)

        # softmax denominator: 4 in-place fp32 Pool add chains over the 16 pb
        # tiles (chain adds emitted inside the exp loop so Pool trails the
        # exps), then a cross-chain combine + partition_all_reduce(add) + DVE
        # reciprocal — no PE or DVE bulk work.
        den_accs = {}

        def emit_denom_add(sb, ti):
            j, k = divmod(ti, 4)
            if k == 0:
                if ti == 0:
                    den_accs[sb] = []
                den_accs[sb].append(
                    p_den.tile([P, SB], FP32, name=f"dac{sb}_{j}", tag=f"dac{j}")
                )
                return
            acc = den_accs[sb][j]
            pbs = pb_tiles[sb]
            if k == 1:
                nc.gpsimd.tensor_add(acc[:], pbs[4 * j][:], pbs[4 * j + 1][:])
            else:
                nc.gpsimd.tensor_add(acc[:], acc[:], pbs[4 * j + k][:])

        def emit_denom_pool(sb):
            accs = den_accs[sb]
            nc.gpsimd.tensor_add(accs[0][:], accs[0][:], accs[1][:])
            nc.gpsimd.tensor_add(accs[2][:], accs[2][:], accs[3][:])
            nc.gpsimd.tensor_add(accs[0][:], accs[0][:], accs[2][:])
            densum = p_aux.tile([P, SB], FP32, name=f"dsum{sb}", tag="dsum")
            nc.gpsimd.partition_all_reduce(
                densum[:], accs[0][:], channels=P,
                reduce_op=bass_isa.ReduceOp.add,
            )
            return densum

        def emit_recip(sb, densum):
            # scheduling hint pins the (Pool-gated) reciprocal BEHIND the
            # window's ot-CASTs in the DVE queue — the scheduler's optimistic
            # Pool model would otherwise hoist it and block the CASTs on HW
            recipb = p_aux.tile([P, SB], FP32, name=f"recipb{sb}", tag="recipb")
            with nc.allow_low_precision(reason="softmax denom reciprocal; fp22 ok"):
                with tc.tile_wait_until(0.060 + 0.080 * sb + 0.050):
                    nc.vector.reciprocal_approx_fast(recipb[:], densum[:])
            return recipb

        def mm4_copy_out(sb, ps_list, ots, et0):
            for i, ps_o in enumerate(ps_list):
                o_t = p_ot.tile([P, SB], OT_DT, name=f"ot{sb}_{et0 + i}", tag="ot")
                nc.vector.tensor_copy(o_t[:], ps_o[:])
                ots.append(o_t)

        def emit_mm5(sb, ots, recipb):
            s0 = sb * SB
            for ft in range(FT):
                ps_f = p_psum1.tile([P, SB], FP32, name=f"psf{sb}_{ft}", tag="ps1")
                for ec in range(DC):
                    nc.tensor.matmul(
                        ps_f[:], fcw_sl(ec, ft), ots[ec][:],
                        start=(ec == 0), stop=(ec == DC - 1),
                    )
                tmp = p_tmp.tile([P, SB], FP32, name=f"tmp{sb}_{ft}", tag="tmp")
                nc.vector.tensor_mul(tmp[:], ps_f[:], recipb[:])
                o_out = p_oo.tile([P, SB], FP32, name=f"oo{sb}_{ft}", tag="oo")
                nc.scalar.activation(
                    o_out[:], tmp[:], AF.Relu,
                    bias=fcb_cols[:, ft : ft + 1], scale=1.0,
                )
                nc.sync.dma_start(outt_d[ft * P : (ft + 1) * P, s0 : s0 + SB], o_out[:])

        # ---------- steady-state blocks ----------
        # MM5(1)/MM5(2) are deferred one window: windows 2 and 3 have no MM1
        # to buffer the PE between the interleave and MM4/exp, so the deferred
        # MM5 fills that slot and absorbs the exp tail
        maxbs[0] = emit_maxb(0)
        pend5 = None
        for sb in range(NSB):
            if sb + 2 < NSB:
                x2t_tiles[sb + 2] = issue_x2t(sb + 2)
            maxb = maxbs[sb]

            if sb + 1 < NSB:
                # PE chews MM2(sb+1) while Scalar/DVE run exp(sb) and Pool
                # trails the denominator adds
                for ti in range(TC):
                    emit_exp_tile(sb, ti, maxb)
                    emit_mm2_tile(sb + 1, ti)
                    emit_denom_add(sb, ti)
                # Pool: denominator combine + allreduce first (recip/MM5 are
                # nearer deadlines than next window's exp needing maxb)
                densum = emit_denom_pool(sb)
                maxbs[sb + 1] = emit_maxb(sb + 1)
                if pend5 is not None:
                    emit_mm5(*pend5)
                    pend5 = None
                if sb + 2 < NSB:
                    emit_mm1(sb + 2)
                ots = []
                for et in range(DC):
                    # first four chains recycle ps1 banks (freed by tt copies /
                    # the deferred MM5's muls, earlier than MM2's sc copies)
                    pp = p_psum1 if et < 4 else p_psum
                    ps_o = pp.tile(
                        [P, SB], FP32, name=f"pso{sb}_{et}",
                        tag="ps1" if et < 4 else "ps",
                    )
                    for ti in range(TC):
                        nc.tensor.matmul(
                            ps_o[:], x1_sl(ti, et), pb_tiles[sb][ti][:],
                            start=(ti == 0), stop=(ti == TC - 1),
                        )
                    mm4_copy_out(sb, [ps_o], ots, et)
                recipb = emit_recip(sb, densum)
                if sb == 0:
                    emit_mm5(sb, ots, recipb)
                else:
                    pend5 = (sb, ots, recipb)
            else:
                # last block: deferred MM5(2) first (PE buffer while exp(3)
                # warms up), then ti-outer MM4 on 7 held banks + a PE rank-1
                # denominator bank (Pool's allreduce is too slow for the tail);
                # et7 chain after, recip broadcast via a rank-1 matmul
                if pend5 is not None:
                    emit_mm5(*pend5)
                    pend5 = None
                mm4_ps = [
                    p_psum1.tile([P, SB], FP32, name=f"pso{sb}_{et}", tag="ps1")
                    for et in range(4)
                ] + [
                    p_psum.tile([P, SB], FP32, name=f"pso{sb}_{et}", tag="ps")
                    for et in range(4, DC - 1)
                ]
                ps_sum = p_psum.tile([1, SB], FP32, name=f"psum{sb}", tag="ps")
                for ti in range(TC):
                    emit_exp_tile(sb, ti, maxb)
                    for et in range(DC - 1):
                        nc.tensor.matmul(
                            mm4_ps[et][:], x1_sl(ti, et), pb_tiles[sb][ti][:],
                            start=(ti == 0), stop=(ti == TC - 1),
                        )
                    nc.tensor.matmul(
                        ps_sum[:], ones_colb[:], pb_tiles[sb][ti][:],
                        start=(ti == 0), stop=(ti == TC - 1),
                    )
                rrow = p_den.tile([1, SB], FP32, name=f"rrow{sb}", tag="rrow")
                with nc.allow_low_precision(reason="softmax denom recip; fp22 ok"):
                    nc.vector.reciprocal_approx_fast(rrow[:], ps_sum[:])
                ots = []
                mm4_copy_out(sb, mm4_ps, ots, 0)
                ps_o7 = p_psum.tile([P, SB], FP32, name=f"pso{sb}_7", tag="ps")
                for ti in range(TC):
                    nc.tensor.matmul(
                        ps_o7[:], x1_sl(ti, DC - 1), pb_tiles[sb][ti][:],
                        start=(ti == 0), stop=(ti == TC - 1),
                    )
                ps_rb = p_psum1.tile([P, SB], FP32, name=f"prb{sb}", tag="ps1")
                nc.tensor.matmul(ps_rb[:], ones_row32[:], rrow[:], start=True, stop=True)
                recipb = p_aux.tile([P, SB], FP32, name=f"recipb{sb}", tag="recipb")
                nc.vector.tensor_copy(recipb[:], ps_rb[:])
                mm4_copy_out(sb, [ps_o7], ots, DC - 1)
                emit_mm5(sb, ots, recipb)

    nc.compile()
    return nc


_NC_CACHE = None


def _get_nc():
    global _NC_CACHE
    if _NC_CACHE is None:
        _NC_CACHE = build_nc()
    return _NC_CACHE


def make_in_maps(x1, x2, U, bias, fc_w, fc_b):
    x1 = np.ascontiguousarray(np.asarray(x1, dtype=np.float32))
    x2 = np.ascontiguousarray(np.asarray(x2, dtype=np.float32))
    U = np.ascontiguousarray(np.asarray(U, dtype=np.float32))
    bias = np.asarray(bias, dtype=np.float32)
    fc_w = np.asarray(fc_w, dtype=np.float32)
    fc_b = np.asarray(fc_b, dtype=np.float32)
    # relaid residents (same for every core)
    ug = np.ascontiguousarray(
        U.reshape(DC, P, D).transpose(1, 0, 2).reshape(P, DC * D)
    ).astype(np.float16)
    fcwg = np.ascontiguousarray(
        fc_w.T.reshape(DC, P, F).transpose(1, 0, 2).reshape(P, DC * F)
    ).astype(np.float16)
    biasg = np.ascontiguousarray(bias.reshape(DC, P).T)
    fcbg = np.ascontiguousarray(fc_b.reshape(FT, P).T)
    in_maps = []
    for b in range(B):
        x1t = x1[b].T  # [D, S]
        x1tg = np.ascontiguousarray(
            x1t.reshape(DC, P, NTG, SB).transpose(1, 2, 0, 3).reshape(P, NTG * DC * SB)
        ).astype(np.float16)
        x1g = np.ascontiguousarray(
            x1[b].reshape(TC, P, D).transpose(1, 0, 2).reshape(P, TC * D)
        ).astype(np.float16)
        in_maps.append(
            {
                "x1g": x1g,
                "x1tg": x1tg,
                "x2t": np.ascontiguousarray(x2[b].T).astype(np.float16),
                "ug": ug,
                "fcwg": fcwg,
                "biasg": biasg,
                "fcbg": fcbg,
            }
        )
    return in_maps


def kernel(x1, x2, U, bias, fc_w, fc_b):
    from concourse.bass_utils import run_bass_kernel_spmd

    nc = _get_nc()
    in_maps = make_in_maps(x1, x2, U, bias, fc_w, fc_b)
    res = run_bass_kernel_spmd(nc, in_maps, core_ids=list(range(B)))
    out = np.stack([np.ascontiguousarray(r["outt"].T) for r in res.results])
    return out.astype(np.float32)


# revision 19
# speedup vs baseline: 1.1972x; 1.1972x over previous
"""BiaffineAttn Trainium2 kernel.

Math (per batch b):
    t    = x2 @ U + bias[None, :]      [S, D]   (bias folded: x2 U x1^T + 1 (x1 bias)^T
                                                 == [x2|1] [[U];[bias^T]] x1^T)
    attn = t @ x1^T
    p    = softmax(attn, axis=-1)
    out  = relu((p @ x1) @ fc_w^T + fc_b)       [S, F]

Sharding: data-parallel over batch B=8, one batch per NeuronCore.

Per-core pipeline, transposed orientation (softmax key dim t' on partitions),
software-pipelined across S-superblocks so the PE runs pure GEMM streams only:
  PE:     MM1 / MM2 / MM4 / MM5 — 352 x 512-col fp16 streams per superblock,
          no transposes, no rank-1 sums, no broadcast matmuls
  Scalar: exp(sb, tile) + the MM2 psum->SBUF score copy (+ MM1 tt copies)
  DVE:    score running-max + maxb subtract, reciprocal, psum->SBUF copies
  Pool:   softmax partition stats — per-s max via partition_all_reduce(max),
          denominator via in-place fp32 add chains over the 16 pb tiles +
          partition_all_reduce(add); both off the PE critical path

All 16-bit matmuls run in fp16 (TRN2 forbids mixing 16/32-bit matmul inputs;
fp16 keeps score noise ~2^-11-scale and LDWEIGHTS at 97ns so the stationary
load hides under the 213ns stream). reciprocal_approx_fast for the denom;
relu+bias via Scalar activation; [F,SB] stores per superblock.

All tensors are host-relaid so every resident loads in ONE big DMA (32KB
rows); x1t t'-groups go on the Pool SWDGE queue (idle during the prologue)
so the scalar HWDGE queue carries only the x2t stream.

Host side: builds the relaid views and transposes the [F,S] per-core output
back to [S,F] when gathering (fp32 DMA transpose does not exist on TRN2).
"""

import os
import sys
from contextlib import ExitStack

import numpy as np

for _p in ("/opt/trn_rl_repo", os.path.expanduser("~/.axon_site/_ro/trn_rl_repo")):
    if os.path.isdir(_p) and _p not in sys.path:
        sys.path.insert(0, _p)

import concourse.bass as bass
import concourse.mybir as mybir
import concourse.tile as tile
from concourse import bacc, bass_isa

B = 8
S = 2048          # sequence length (both s and t')
D = 1024          # d_model
F = 512           # fc output dim
P = 128
SB = 512          # s superblock (moving free dim of every matmul)
NSB = S // SB     # 4
DC = D // P       # 8 contraction chunks of d / e
TC = S // P       # 16 t' tiles
NTG = NSB         # 4 t' groups of 4 tiles
FT = F // P       # 4
FP32 = mybir.dt.float32
BF16 = mybir.dt.bfloat16
FP16 = mybir.dt.float16
AF = mybir.ActivationFunctionType
ALU = mybir.AluOpType
AX = mybir.AxisListType

OT_DT = FP16      # dtype of MM4 output tiles (MM5 moving operand)


def build_nc():
    nc = bacc.Bacc(
        "TRN2",
        target_bir_lowering=False,
        debug=False,
        enable_asserts=False,
    )

    # host-relaid tensors: row p holds the p-th partition's data for every tile
    x1_d = nc.dram_tensor("x1g", [P, TC * D], FP16, kind="ExternalInput")
    x1t_d = nc.dram_tensor("x1tg", [P, NTG * DC * SB], FP16, kind="ExternalInput")
    x2t_d = nc.dram_tensor("x2t", [D, S], FP16, kind="ExternalInput")
    u_d = nc.dram_tensor("ug", [P, DC * D], FP16, kind="ExternalInput")
    fcwt_d = nc.dram_tensor("fcwg", [P, DC * F], FP16, kind="ExternalInput")
    bias_d = nc.dram_tensor("biasg", [P, DC], FP32, kind="ExternalInput")
    fcb_d = nc.dram_tensor("fcbg", [P, FT], FP32, kind="ExternalInput")
    outt_d = nc.dram_tensor("outt", [F, S], FP32, kind="ExternalOutput")

    with tile.TileContext(nc) as tc, ExitStack() as ctx:
        # ---------- pools ----------
        p_u = ctx.enter_context(tc.tile_pool(name="ures", bufs=1))
        p_x1 = ctx.enter_context(tc.tile_pool(name="x1res", bufs=1))
        p_bc = ctx.enter_context(tc.tile_pool(name="biascols", bufs=1))
        p_fcb = ctx.enter_context(tc.tile_pool(name="fcbcols", bufs=1))
        p_fcw = ctx.enter_context(tc.tile_pool(name="fcwres", bufs=1))
        p_psum = ctx.enter_context(tc.tile_pool(name="psum", bufs=4, space="PSUM"))
        p_psum1 = ctx.enter_context(tc.tile_pool(name="psum1", bufs=4, space="PSUM"))
        p_x2t = ctx.enter_context(tc.tile_pool(name="x2ts", bufs=12))
        p_x1tc = ctx.enter_context(tc.tile_pool(name="x1tgs", bufs=4))
        p_tt = ctx.enter_context(tc.tile_pool(name="tts", bufs=9))
        p_sc = ctx.enter_context(tc.tile_pool(name="scores", bufs=TC))
        p_pb = ctx.enter_context(tc.tile_pool(name="pbf", bufs=TC))
        p_ot = ctx.enter_context(tc.tile_pool(name="ots", bufs=12))
        p_aux = ctx.enter_context(tc.tile_pool(name="aux", bufs=1))
        p_den = ctx.enter_context(tc.tile_pool(name="dens", bufs=1))
        p_tmp = ctx.enter_context(tc.tile_pool(name="tmps", bufs=2))
        p_oo = ctx.enter_context(tc.tile_pool(name="oos", bufs=4))

        # ---------- prologue DMAs: MM1(0) critical path first ----------
        # x1t resident (4MB fp16 = the whole tensor): one SWDGE DMA per
        # t'-group on the otherwise-idle Pool queue, so the scalar HWDGE
        # queue carries only the x2t stream and tg0 lands before MM2(0) ti0
        x1tg = {}

        def issue_x1tg(key, tg, eng=None):
            eng = eng or nc.gpsimd
            t = p_x1tc.tile([P, DC * SB], FP16, name=f"x1tg{key}", tag="x1tg")
            eng.dma_start(t[:], x1t_d[:, tg * DC * SB : (tg + 1) * DC * SB])
            x1tg[key] = t

        # per-dc pieces: each lands just ahead of MM1(0)'s dc step;
        # tg2/tg3 queue on sync behind u (deadlines MM2(0) ti8/ti12)
        u_big = p_u.tile([P, DC * D], FP16, name="ug", tag="ur")
        for q in range(4):
            nc.sync.dma_start(
                u_big[:, q * (D // 4) : (q + 1) * (D // 4)],
                u_d[:, q * (D // 4) : (q + 1) * (D // 4)],
            )
        for dc in range(1, DC):
            nc.sync.dma_start(
                u_big[:, dc * D : (dc + 1) * D], u_d[:, dc * D : (dc + 1) * D]
            )
        issue_x1tg(0, 0, nc.sync)
        issue_x1tg(1, 1, nc.sync)
        issue_x1tg(2, 2, nc.sync)
        issue_x1tg(3, 3, nc.sync)

        def u_sl(dc, et):
            return u_big[:, dc * D + et * P : dc * D + (et + 1) * P]

        # scalar HWDGE queue: x2t(0) then bias then x2t(1)
        bias_cols = p_bc.tile([P, DC], FP32, name="bc", tag="bc")
        nc.scalar.dma_start(bias_cols[:], bias_d[:, :])
        x2t_tiles = {0: []}
        for dc in range(DC):
            x2_t = p_x2t.tile([P, SB], FP16, name=f"x2t0_{dc}", tag="x2t")
            nc.scalar.dma_start(x2_t[:], x2t_d[dc * P : (dc + 1) * P, 0:SB])
            x2t_tiles[0].append(x2_t)

        def issue_x2t(sb, eng=None):
            eng = eng or nc.sync
            tiles = []
            for dc in range(DC):
                x2_t = p_x2t.tile([P, SB], FP16, name=f"x2t{sb}_{dc}", tag="x2t")
                eng.dma_start(
                    x2_t[:], x2t_d[dc * P : (dc + 1) * P, sb * SB : (sb + 1) * SB]
                )
                tiles.append(x2_t)
            return tiles

        x2t_tiles[1] = issue_x2t(1, nc.scalar)

        # residents declared here; DMAs issued after prologue compute emission
        # (needed only from MM4(0)/MM5(0) on — keep them off the critical
        # 20-40us DMA window)
        x1_big = p_x1.tile([P, TC * D], FP16, name="x1g", tag="x1r")

        def x1_sl(ti, et):
            return x1_big[:, ti * D + et * P : ti * D + (et + 1) * P]

        fcw_big = p_fcw.tile([P, DC * F], FP16, name="fcwg", tag="fcw")

        def fcw_sl(ec, ft):
            return fcw_big[:, ec * F + ft * P : ec * F + (ft + 1) * P]

        fcb_cols = p_fcb.tile([P, FT], FP32, name="fcb", tag="fcb")

        ones_colb = p_bc.tile([P, 1], FP16, name="ones_colb", tag="ones_c")
        nc.gpsimd.memset(ones_colb[:], 1.0)
        ones_row32 = p_bc.tile([1, P], FP32, name="ones_row32", tag="ones_r")
        nc.gpsimd.memset(ones_row32[:], 1.0)

        # ---------- MM1: ttT = (x2 @ U)^T + bias, 8 held banks (both pools) ----------
        tt = {}

        def emit_mm1(sb, streaming=False, dve_copies=False):
            tt[sb] = []
            if streaming:
                # dc-outer, 8 held banks: consumes each u/x2t piece once as it
                # lands (prologue is DMA-paced)
                ps = [
                    p_psum1.tile([P, SB], FP32, name=f"ps1_{sb}_{et}", tag="ps1")
                    for et in range(4)
                ] + [
                    p_psum.tile([P, SB], FP32, name=f"ps1b_{sb}_{et}", tag="ps")
                    for et in range(4, DC)
                ]
                for dc in range(DC):
                    for et in range(DC):
                        nc.tensor.matmul(
                            ps[et][:], u_sl(dc, et), x2t_tiles[sb][dc][:],
                            start=(dc == 0), stop=(dc == DC - 1),
                        )
                for et in range(DC):
                    t_t = p_tt.tile([P, SB], FP16, name=f"tt{sb}_{et}", tag="tt")
                    nc.scalar.activation(
                        t_t[:], ps[et][:], AF.Identity,
                        bias=bias_cols[:, et : et + 1], scale=1.0,
                    )
                    tt[sb].append(t_t)
                return
            # et-outer, one rotating bank: copies spread across MM1 so the
            # scalar queue never piles up at the block seam
            for et in range(DC):
                ps = p_psum1.tile([P, SB], FP32, name=f"ps1_{sb}_{et}", tag="ps1")
                for dc in range(DC):
                    nc.tensor.matmul(
                        ps[:], u_sl(dc, et), x2t_tiles[sb][dc][:],
                        start=(dc == 0), stop=(dc == DC - 1),
                    )
                t_t = p_tt.tile([P, SB], FP16, name=f"tt{sb}_{et}", tag="tt")
                if dve_copies:
                    # DVE bias-fused copy: keeps the scalar queue free for the
                    # surrounding MM2 sc-copies (prologue-only, avoids deadlock)
                    nc.vector.tensor_scalar_add(
                        t_t[:], ps[:], bias_cols[:, et : et + 1]
                    )
                else:
                    nc.scalar.activation(
                        t_t[:], ps[:], AF.Identity,
                        bias=bias_cols[:, et : et + 1], scale=1.0,
                    )
                tt[sb].append(t_t)

        emit_mm1(0, streaming=True)

        # ---------- MM2 per-tile emitter: scoresT tile + running max ----------
        sc_tiles = {}
        maxacc = {}
        pb_tiles = {}

        def emit_mm2_tile(sb, ti):
            tg, sub = divmod(ti, SB // P)
            if ti == 0:
                sc_tiles[sb] = []
                maxacc[sb] = p_aux.tile(
                    [P, SB], FP32, name=f"maxacc{sb}", tag=f"maxacc{sb % 2}"
                )
            ps_s = p_psum.tile([P, SB], FP32, name=f"pss{sb}_{ti}", tag="ps")
            grp = x1tg[tg]
            for ec in range(DC):
                nc.tensor.matmul(
                    ps_s[:],
                    grp[:, ec * SB + sub * P : ec * SB + (sub + 1) * P],
                    tt[sb][ec][:],
                    start=(ec == 0), stop=(ec == DC - 1),
                )
            s_t = p_sc.tile([P, SB], FP32, name=f"sc{sb}_{ti}", tag="sc")
            nc.scalar.copy(s_t[:], ps_s[:])
            if ti == 0:
                nc.vector.tensor_copy(maxacc[sb][:], s_t[:])
            else:
                nc.vector.tensor_max(maxacc[sb][:], maxacc[sb][:], s_t[:])
            sc_tiles[sb].append(s_t)

        # prologue MM2(0) tg0, then MM1(1) (tt copies on DVE so the scalar
        # queue keeps draining MM2 sc-copies), then MM2(0) tg1-3: pushes the
        # tg1-3 DMA deadlines out by MM1(1)'s 14us
        for ti in range(4):
            emit_mm2_tile(0, ti)
        emit_mm1(1, dve_copies=True)
        for ti in range(4, TC):
            emit_mm2_tile(0, ti)
        nc.sync.dma_start(x1_big[:, : TC * D // 2], x1_d[:, : TC * D // 2])
        nc.sync.dma_start(x1_big[:, TC * D // 2 :], x1_d[:, TC * D // 2 :])
        nc.sync.dma_start(fcw_big[:], fcwt_d[:, :])
        nc.sync.dma_start(fcb_cols[:], fcb_d[:, :])
        maxbs = {}

        # ---------- softmax helpers (all partition stats on Pool) ----------
        def emit_maxb(sb):
            """per-s max over partitions, broadcast back: one Pool op."""
            maxb = p_aux.tile([P, SB], FP32, name=f"maxb{sb}", tag=f"maxb{sb % 2}")
            nc.gpsimd.partition_all_reduce(
                maxb[:], maxacc[sb][:], channels=P,
                reduce_op=bass_isa.ReduceOp.max,
            )
            return maxb

        def emit_exp_tile(sb, ti, maxb):
            if ti == 0:
                pb_tiles[sb] = []
            s_t = sc_tiles[sb][ti]
            # scheduling hint only (no HW wait): keep window sb's subs behind
            # window sb-1's ot-CASTs in the DVE queue — the list scheduler's
            # Pool cost model is optimistic and would otherwise hoist them,
            # stalling MM4's PSUM-bank recycling on real hardware
            with tc.tile_wait_until(0.060 + 0.080 * sb):
                nc.vector.tensor_sub(s_t[:], s_t[:], maxb[:])
            p_t = p_pb.tile([P, SB], FP16, name=f"pb{sb}_{ti}", tag="pb")
            nc.scalar.activation(p_t[:], s_t[:], AF.Exp, bias=0.0, scale=1.0)
            pb_tiles[sb].append(p_t)

        # softmax denominator: 4 in-place fp32 Pool add chains over the 16 pb
        # tiles (chain adds emitted inside the exp loop so Pool trails the
        # exps), then a cross-chain combine + partition_all_reduce(add) + DVE
        # reciprocal — no PE or DVE bulk work.
        den_accs = {}

        def emit_denom_add(sb, ti):
            j, k = divmod(ti, 4)
            if k == 0:
                if ti == 0:
                    den_accs[sb] = []
                den_accs[sb].append(
                    p_den.tile([P, SB], FP32, name=f"dac{sb}_{j}", tag=f"dac{j}")
                )
                return
            acc = den_accs[sb][j]
            pbs = pb_tiles[sb]
            if k == 1:
                nc.gpsimd.tensor_add(acc[:], pbs[4 * j][:], pbs[4 * j + 1][:])
            else:
                nc.gpsimd.tensor_add(acc[:], acc[:], pbs[4 * j + k][:])

        def emit_denom_pool(sb):
            accs = den_accs[sb]
            nc.gpsimd.tensor_add(accs[0][:], accs[0][:], accs[1][:])
            nc.gpsimd.tensor_add(accs[2][:], accs[2][:], accs[3][:])
            nc.gpsimd.tensor_add(accs[0][:], accs[0][:], accs[2][:])
            densum = p_aux.tile([P, SB], FP32, name=f"dsum{sb}", tag="dsum")
            nc.gpsimd.partition_all_reduce(
                densum[:], accs[0][:], channels=P,
                reduce_op=bass_isa.ReduceOp.add,
            )
            return densum

        def emit_recip(sb, densum):
            # scheduling hint pins the (Pool-gated) reciprocal BEHIND the
            # window's ot-CASTs in the DVE queue — the scheduler's optimistic
            # Pool model would otherwise hoist it and block the CASTs on HW
            recipb = p_aux.tile([P, SB], FP32, name=f"recipb{sb}", tag="recipb")
            with nc.allow_low_precision(reason="softmax denom reciprocal; fp22 ok"):
                with tc.tile_wait_until(0.060 + 0.080 * sb + 0.050):
                    nc.vector.reciprocal_approx_fast(recipb[:], densum[:])
            return recipb

        def mm4_copy_out(sb, ps_list, ots, et0):
            for i, ps_o in enumerate(ps_list):
                o_t = p_ot.tile([P, SB], OT_DT, name=f"ot{sb}_{et0 + i}", tag="ot")
                nc.vector.tensor_copy(o_t[:], ps_o[:])
                ots.append(o_t)

        def emit_mm5(sb, ots, recipb, relu_wait=None):
            s0 = sb * SB
            for ft in range(FT):
                ps_f = p_psum1.tile([P, SB], FP32, name=f"psf{sb}_{ft}", tag="ps1")
                for ec in range(DC):
                    nc.tensor.matmul(
                        ps_f[:], fcw_sl(ec, ft), ots[ec][:],
                        start=(ec == 0), stop=(ec == DC - 1),
                    )
                tmp = p_tmp.tile([P, SB], FP32, name=f"tmp{sb}_{ft}", tag="tmp")
                nc.vector.tensor_mul(tmp[:], ps_f[:], recipb[:])
                o_out = p_oo.tile([P, SB], FP32, name=f"oo{sb}_{ft}", tag="oo")
                # deferred MM5: hint the relu behind the next window's early
                # exps in the scalar queue (the store has a whole window of
                # slack; the exps gate MM4's moving operands)
                with tc.tile_wait_until(relu_wait or 0.0, enable=relu_wait is not None):
                    nc.scalar.activation(
                        o_out[:], tmp[:], AF.Relu,
                        bias=fcb_cols[:, ft : ft + 1], scale=1.0,
                    )
                nc.sync.dma_start(outt_d[ft * P : (ft + 1) * P, s0 : s0 + SB], o_out[:])

        # ---------- steady-state blocks ----------
        # MM5(1)/MM5(2) are deferred one window: windows 2 and 3 have no MM1
        # to buffer the PE between the interleave and MM4/exp, so the deferred
        # MM5 fills that slot and absorbs the exp tail
        maxbs[0] = emit_maxb(0)
        pend5 = None
        for sb in range(NSB):
            if sb + 2 < NSB:
                x2t_tiles[sb + 2] = issue_x2t(sb + 2)
            maxb = maxbs[sb]

            if sb + 1 < NSB:
                # PE chews MM2(sb+1) while Scalar/DVE run exp(sb) and Pool
                # trails the denominator adds
                for ti in range(TC):
                    emit_exp_tile(sb, ti, maxb)
                    emit_mm2_tile(sb + 1, ti)
                    emit_denom_add(sb, ti)
                # Pool: denominator combine + allreduce first (recip/MM5 are
                # nearer deadlines than next window's exp needing maxb)
                densum = emit_denom_pool(sb)
                maxbs[sb + 1] = emit_maxb(sb + 1)
                if pend5 is not None:
                    emit_mm5(*pend5, relu_wait=0.060 + 0.080 * sb + 0.010)
                    pend5 = None
                if sb + 2 < NSB:
                    emit_mm1(sb + 2)
                ots = []
                for et in range(DC):
                    # first four chains recycle ps1 banks (freed by tt copies /
                    # the deferred MM5's muls, earlier than MM2's sc copies)
                    pp = p_psum1 if et < 4 else p_psum
                    ps_o = pp.tile(
                        [P, SB], FP32, name=f"pso{sb}_{et}",
                        tag="ps1" if et < 4 else "ps",
                    )
                    for ti in range(TC):
                        nc.tensor.matmul(
                            ps_o[:], x1_sl(ti, et), pb_tiles[sb][ti][:],
                            start=(ti == 0), stop=(ti == TC - 1),
                        )
                    mm4_copy_out(sb, [ps_o], ots, et)
                recipb = emit_recip(sb, densum)
                if sb == 0:
                    emit_mm5(sb, ots, recipb)
                else:
                    pend5 = (sb, ots, recipb)
            else:
                # last block: deferred MM5(2) first (PE buffer while exp(3)
                # warms up), then ti-outer MM4 on 7 held banks + a PE rank-1
                # denominator bank (Pool's allreduce is too slow for the tail);
                # et7 chain after, recip broadcast via a rank-1 matmul
                if pend5 is not None:
                    emit_mm5(*pend5, relu_wait=0.060 + 0.080 * sb + 0.010)
                    pend5 = None
                mm4_ps = [
                    p_psum1.tile([P, SB], FP32, name=f"pso{sb}_{et}", tag="ps1")
                    for et in range(4)
                ] + [
                    p_psum.tile([P, SB], FP32, name=f"pso{sb}_{et}", tag="ps")
                    for et in range(4, DC - 1)
                ]
                ps_sum = p_psum.tile([1, SB], FP32, name=f"psum{sb}", tag="ps")
                for ti in range(TC):
                    emit_exp_tile(sb, ti, maxb)
                    for et in range(DC - 1):
                        nc.tensor.matmul(
                            mm4_ps[et][:], x1_sl(ti, et), pb_tiles[sb][ti][:],
                            start=(ti == 0), stop=(ti == TC - 1),
                        )
                    nc.tensor.matmul(
                        ps_sum[:], ones_colb[:], pb_tiles[sb][ti][:],
                        start=(ti == 0), stop=(ti == TC - 1),
                    )
                rrow = p_den.tile([1, SB], FP32, name=f"rrow{sb}", tag="rrow")
                with nc.allow_low_precision(reason="softmax denom recip; fp22 ok"):
                    nc.vector.reciprocal_approx_fast(rrow[:], ps_sum[:])
                ots = []
                mm4_copy_out(sb, mm4_ps, ots, 0)
                ps_o7 = p_psum.tile([P, SB], FP32, name=f"pso{sb}_7", tag="ps")
                for ti in range(TC):
                    nc.tensor.matmul(
                        ps_o7[:], x1_sl(ti, DC - 1), pb_tiles[sb][ti][:],
                        start=(ti == 0), stop=(ti == TC - 1),
                    )
                ps_rb = p_psum1.tile([P, SB], FP32, name=f"prb{sb}", tag="ps1")
                nc.tensor.matmul(ps_rb[:], ones_row32[:], rrow[:], start=True, stop=True)
                recipb = p_aux.tile([P, SB], FP32, name=f"recipb{sb}", tag="recipb")
                nc.vector.tensor_copy(recipb[:], ps_rb[:])
                mm4_copy_out(sb, [ps_o7], ots, DC - 1)
                emit_mm5(sb, ots, recipb)

    nc.compile()
    return nc


_NC_CACHE = None


def _get_nc():
    global _NC_CACHE
    if _NC_CACHE is None:
        _NC_CACHE = build_nc()
    return _NC_CACHE


def make_in_maps(x1, x2, U, bias, fc_w, fc_b):
    x1 = np.ascontiguousarray(np.asarray(x1, dtype=np.float32))
    x2 = np.ascontiguousarray(np.asarray(x2, dtype=np.float32))
    U = np.ascontiguousarray(np.asarray(U, dtype=np.float32))
    bias = np.asarray(bias, dtype=np.float32)
    fc_w = np.asarray(fc_w, dtype=np.float32)
    fc_b = np.asarray(fc_b, dtype=np.float32)
    # relaid residents (same for every core)
    ug = np.ascontiguousarray(
        U.reshape(DC, P, D).transpose(1, 0, 2).reshape(P, DC * D)
    ).astype(np.float16)
    fcwg = np.ascontiguousarray(
        fc_w.T.reshape(DC, P, F).transpose(1, 0, 2).reshape(P, DC * F)
    ).astype(np.float16)
    biasg = np.ascontiguousarray(bias.reshape(DC, P).T)
    fcbg = np.ascontiguousarray(fc_b.reshape(FT, P).T)
    in_maps = []
    for b in range(B):
        x1t = x1[b].T  # [D, S]
        x1tg = np.ascontiguousarray(
            x1t.reshape(DC, P, NTG, SB).transpose(1, 2, 0, 3).reshape(P, NTG * DC * SB)
        ).astype(np.float16)
        x1g = np.ascontiguousarray(
            x1[b].reshape(TC, P, D).transpose(1, 0, 2).reshape(P, TC * D)
        ).astype(np.float16)
        in_maps.append(
            {
                "x1g": x1g,
                "x1tg": x1tg,
                "x2t": np.ascontiguousarray(x2[b].T).astype(np.float16),
                "ug": ug,
                "fcwg": fcwg,
                "biasg": biasg,
                "fcbg": fcbg,
            }
        )
    return in_maps


def kernel(x1, x2, U, bias, fc_w, fc_b):
    from concourse.bass_utils import run_bass_kernel_spmd

    nc = _get_nc()
    in_maps = make_in_maps(x1, x2, U, bias, fc_w, fc_b)
    res = run_bass_kernel_spmd(nc, in_maps, core_ids=list(range(B)))
    out = np.stack([np.ascontiguousarray(r["outt"].T) for r in res.results])
    return out.astype(np.float32)
